# revision 21
# baseline (speedup 1.0000x reference)
"""Trainium2 Bass kernel for a 2-branch GCN (gcn_conv -> leaky_relu -> mean-pool -> fc
head), data-parallel over graphs across 8 NeuronCores.

Math: gcn_conv(x, A, W) = (A_norm @ x) @ W + b. We aggregate raw node features first
(gather rows straight from DRAM), then do the dense 1024x1024 matmul on each core's
node shard only. dinv[src]*dinv[dst] factorizes: x rows are pre-scaled by dinv on the
host, aggregated rows are scaled by dinv[dst] on the device; the self-loop (1/deg)
folds in as a plain self-edge.

Quantization: the gathered features are fp8 to halve the dominant HBM gather
traffic. Features 0-511 are stored as fp8 e4m3 and aggregated with DoubleRow
matmuls (2 k-subtiles per instruction); features 512-1023 are fp8 e3m4 (one more
mantissa bit) aggregated at the normal rate. The one-hot lhsT is built once in
e4m3 and shared (mixed e4 lhsT x e3 rhs is supported). Everything downstream of
the aggregation (W matmul, pooling, fc heads) stays bf16/f32.

Sharding: core c owns graphs [8c, 8c+8) (batch is sorted, so its nodes are one
contiguous range). Edges are routed by dst. The per-window flow is software-
pipelined four stages deep (gather -> aggregate -> transpose -> W/pool) so the
TensorEngine never waits on DMA or PSUM evictions.
"""

import os
import sys

for _p in ("/opt/trn_rl_repo",):
    if _p not in sys.path:
        sys.path.insert(0, _p)

import numpy as np
import ml_dtypes

import concourse.bacc as bacc
import concourse.mybir as mybir
import concourse.tile as tile
from concourse import bass_utils
from concourse.masks import make_identity

N, E, B, D, OUT_D = 10000, 160000, 64, 1024, 128
NCORES = 8
GPC = B // NCORES  # graphs per core
P = 128
NJ = D // P  # feature chunks (8)
HALF = D // 2

BF16 = ml_dtypes.bfloat16
E4M3 = ml_dtypes.float8_e4m3
E3M4 = ml_dtypes.float8_e3m4

TRACE = [False]  # test.py can flip this for profiled runs
LAST_RESULTS = [None]

_IOTAM = np.ascontiguousarray(
    np.tile(np.arange(128, dtype=np.float32)[None, :], (128, 1)))


def _mode():
    return os.environ.get("K_GDT", "split")  # split | e3 | bf16


def _pack_xs(xsf):
    """[N, D] f32 (dinv-scaled) -> packed DRAM payload per K_GDT."""
    mode = _mode()
    if mode == "bf16":
        return np.ascontiguousarray(xsf.astype(BF16))
    if mode == "e3":
        return np.ascontiguousarray(xsf.astype(E3M4))
    a = xsf[:, :HALF].astype(E4M3)
    b = xsf[:, HALF:].astype(E3M4)
    packed = np.empty((N, D), np.uint8)
    packed[:, :HALF] = a.view(np.uint8)
    packed[:, HALF:] = b.view(np.uint8)
    return packed.view(E4M3)


def _prep_branch(x, edge_index, batch):
    """Host-side index preprocessing + array layout for one GCN branch.

    Returns (meta, percore) where meta = dict(nwin, cpw, off, totch) and
    percore[c] = dict of arrays named without branch suffix.
    """
    x = np.asarray(x, np.float32)
    src = np.asarray(edge_index[0], np.int64)
    dst = np.asarray(edge_index[1], np.int64)
    batch = np.asarray(batch, np.int64)

    cnt = np.bincount(batch, minlength=B)
    gstart = np.zeros(B + 1, np.int64)
    gstart[1:] = np.cumsum(cnt)

    deg = np.bincount(dst, minlength=N).astype(np.float32) + 1.0
    dinv = (1.0 / np.sqrt(deg)).astype(np.float32)
    xs = _pack_xs(x * dinv[:, None])
    # self-loop term (x[i]/deg[i]) is NOT gathered: it's contiguous rows,
    # streamed separately in bf16 and added during the z eviction
    xself_full = (x * (dinv * dinv)[:, None]).astype(BF16)

    order = np.argsort(dst, kind="stable")
    src_all = src[order]
    dst_all = dst[order]

    cores = np.arange(NCORES)
    n0 = gstart[cores * GPC]
    n1 = gstart[cores * GPC + GPC]
    e0 = np.searchsorted(dst_all, n0, side="left")
    e1 = np.searchsorted(dst_all, n1, side="left")
    nodes_c = n1 - n0
    nwin = int(np.max((nodes_c + P - 1) // P))

    wcounts = np.zeros((NCORES, nwin), np.int64)
    for c in range(NCORES):
        w = (dst_all[e0[c]:e1[c]] - n0[c]) // P
        wcounts[c] = np.bincount(w, minlength=nwin)
    cpw = np.maximum(1, (wcounts.max(axis=0) + P - 1) // P).astype(np.int64)
    off = np.zeros(nwin + 1, np.int64)
    off[1:] = np.cumsum(cpw)
    totch = int(off[-1])

    percore = []
    for c in range(NCORES):
        s_loc = src_all[e0[c]:e1[c]]
        d_loc = dst_all[e0[c]:e1[c]] - n0[c]
        w = d_loc // P
        wstart = np.zeros(nwin + 1, np.int64)
        wstart[1:] = np.cumsum(wcounts[c])
        pos_in_win = np.arange(len(d_loc)) - wstart[w]
        slot = off[w] * P + pos_in_win  # unique edge slot in [0, totch*P)

        src_pad = np.zeros(totch * P, np.int32)
        src_pad[slot] = s_loc

        dl = np.full(totch * P, -1.0, np.float32)
        dl[slot] = (d_loc % P).astype(np.float32)
        dstloc = np.ascontiguousarray(dl.reshape(totch, P).T)  # [128, totch]

        # int16 gather indices, wrapped per *window* (one dma_gather per window):
        # within a call, edge e -> [e % 16, e // 16], replicated to 128 partitions
        idx = np.zeros((16, totch * 8), np.int16)
        for wi in range(nwin):
            sw = src_pad[off[wi] * P: off[wi + 1] * P]
            idx[:, off[wi] * 8: off[wi + 1] * 8] = (
                sw.astype(np.int16).reshape(-1, 16).T
            )
        idx = np.ascontiguousarray(np.tile(idx, (8, 1)))

        ids = (n0[c] + np.arange(nwin * P)).reshape(nwin, P)
        valid = ids < n1[c]
        idc = np.minimum(ids, N - 1)
        dv = np.where(valid, dinv[idc], 0.0).astype(np.float32)
        dinv_col = np.ascontiguousarray(dv.T)  # [128, nwin]

        xself = np.where(valid.reshape(-1)[:, None], xself_full[idc.reshape(-1)],
                         np.zeros((1, D), BF16))
        xself = np.ascontiguousarray(xself.astype(BF16))  # [nwin*P, D]

        bloc = batch[idc] - c * GPC
        pm4 = np.zeros((nwin, P, GPC), np.float32)
        wi_idx, p_idx = np.nonzero(valid)
        pm4[wi_idx, p_idx, bloc[valid]] = 1.0
        pm = np.ascontiguousarray(pm4.transpose(1, 0, 2).reshape(P, nwin * GPC).astype(BF16))

        civ = (1.0 / np.maximum(cnt[c * GPC:(c + 1) * GPC], 1)).astype(np.float32)
        ci = np.ascontiguousarray(civ.reshape(GPC, 1))

        percore.append(
            dict(xs=xs, dl=dstloc, idx=idx, dinv=dinv_col, pm=pm, ci=ci,
                 xself=xself)
        )

    meta = dict(nwin=nwin, cpw=tuple(int(v) for v in cpw),
                off=tuple(int(v) for v in off), totch=totch)
    return meta, percore


def _reshape_w(W):
    # [D, D] -> [128, NJ*D]  with [p, j*D + o] = W[j*128 + p, o]
    W = np.asarray(W, np.float32)
    return np.ascontiguousarray(
        W.reshape(NJ, P, D).transpose(1, 0, 2).reshape(P, NJ * D).astype(BF16)
    )


def _reshape_fc(Wfc):
    # [D, OUT_D] -> [128, NJ*OUT_D] fp32
    Wfc = np.asarray(Wfc, np.float32)
    return np.ascontiguousarray(
        Wfc.reshape(NJ, P, OUT_D).transpose(1, 0, 2).reshape(P, NJ * OUT_D)
    )


_PROGRAM_CACHE = {}


def _build_program(meta0, meta1, has_bias, has_fcb, has_finb):
    mode = _mode()
    f32 = mybir.dt.float32
    bf16 = mybir.dt.bfloat16
    if mode == "bf16":
        gdt = sdt = bf16
        dr = False
    elif mode == "e3":
        gdt = sdt = mybir.dt.float8e3
        dr = False
    else:
        gdt = sdt = mybir.dt.float8e4
        dr = os.environ.get("K_NODR", "") != "1"
    e3 = mybir.dt.float8e3 if mode == "split" else sdt
    esz = 2 if mode == "bf16" else 1  # bytes per gathered element
    Alu = mybir.AluOpType
    Act = mybir.ActivationFunctionType
    DR = mybir.MatmulPerfMode.DoubleRow

    metas = (meta0, meta1)
    maxcw = max(max(meta0["cpw"]), max(meta1["cpw"]))
    nq = int(os.environ.get("K_NQ", "4"))
    nc = bacc.Bacc("TRN2", num_devices=NCORES, debug=False, num_swdge_queues=nq)

    # DRAM tensors
    xs_d, idx_d, dl_d, dinv_d, pm_d, ci_d, W_d, bias_d, fc_d, fcb_d = (
        [], [], [], [], [], [], [], [], [], [])
    xself_d = []
    for b in (0, 1):
        m = metas[b]
        xs_d.append(nc.dram_tensor(f"xs{b}", [N, D], gdt, kind="ExternalInput"))
        idx_d.append(nc.dram_tensor(f"idx{b}", [P, m["totch"] * 8], mybir.dt.int16,
                                    kind="ExternalInput"))
        dl_d.append(nc.dram_tensor(f"dl{b}", [P, m["totch"]], f32,
                                   kind="ExternalInput"))
        dinv_d.append(nc.dram_tensor(f"dinv{b}", [P, m["nwin"]], f32,
                                     kind="ExternalInput"))
        pm_d.append(nc.dram_tensor(f"pm{b}", [P, m["nwin"] * GPC], bf16,
                                   kind="ExternalInput"))
        xself_d.append(nc.dram_tensor(f"xself{b}", [m["nwin"] * P, D], bf16,
                                      kind="ExternalInput"))
        ci_d.append(nc.dram_tensor(f"ci{b}", [GPC, 1], f32, kind="ExternalInput"))
        W_d.append(nc.dram_tensor(f"W{b}", [P, NJ * D], bf16, kind="ExternalInput"))
        fc_d.append(nc.dram_tensor(f"fc{b}", [P, NJ * OUT_D], f32,
                                   kind="ExternalInput"))
        bias_d.append(nc.dram_tensor(f"bias{b}", [1, D], bf16, kind="ExternalInput")
                      if has_bias[b] else None)
        fcb_d.append(nc.dram_tensor(f"fcb{b}", [1, OUT_D], f32, kind="ExternalInput")
                     if has_fcb[b] else None)
    fin_d = nc.dram_tensor("fin", [P, 2], f32, kind="ExternalInput")
    iota_d = nc.dram_tensor("iotam", [P, P], f32, kind="ExternalInput")
    finb_d = nc.dram_tensor("finb", [1, 1], f32, kind="ExternalInput") if has_finb else None
    out_d = nc.dram_tensor("out", [GPC, 1], f32, kind="ExternalOutput")

    with tile.TileContext(nc) as tc:
        with (
            tc.tile_pool(name="const", bufs=1) as cpool,
            tc.tile_pool(name="xg", bufs=3) as xgpool,
            tc.tile_pool(name="ohp", bufs=3) as ohpool,
            tc.tile_pool(name="zsb", bufs=2) as zpool,
            tc.tile_pool(name="ztsb", bufs=2) as ztpool,
            tc.tile_pool(name="ysb", bufs=3) as ypool,
            tc.tile_pool(name="xslf", bufs=3) as xspool,
            tc.tile_pool(name="pz", bufs=1, space="PSUM") as pz,
            tc.tile_pool(name="pzt", bufs=2, space="PSUM") as pzt,
            tc.tile_pool(name="ph", bufs=1, space="PSUM") as ph,
            tc.tile_pool(name="pp", bufs=1, space="PSUM") as pp,
        ):
            identity = cpool.tile([P, P], f32, tag="ident", name="ident")
            make_identity(nc, identity[:])
            identity_bf = cpool.tile([P, P], bf16, tag="identbf", name="identbf")
            nc.vector.tensor_copy(out=identity_bf[:], in_=identity[:])
            iotam = cpool.tile([P, P], f32, tag="iotam", name="iotam")
            nc.sync.dma_start(out=iotam[:], in_=iota_d.ap())

            # resident small/medium tensors
            idx_sb, dl_sb, dinv_sb, pm_sb, ci_sb, bias_sb, fcb_sb = (
                [], [], [], [], [], [], [])
            W_sb, fc_sb = [], []
            for b in (0, 1):
                m = metas[b]
                t = cpool.tile([P, m["totch"] * 8], mybir.dt.int16, tag=f"idx{b}",
                               name=f"idx{b}sb")
                nc.sync.dma_start(out=t[:], in_=idx_d[b].ap())
                idx_sb.append(t)
                t = cpool.tile([P, m["totch"]], f32, tag=f"dl{b}", name=f"dl{b}sb")
                nc.sync.dma_start(out=t[:], in_=dl_d[b].ap())
                dl_sb.append(t)
                t = cpool.tile([P, m["nwin"]], f32, tag=f"dinv{b}", name=f"dinv{b}sb")
                nc.sync.dma_start(out=t[:], in_=dinv_d[b].ap())
                dinv_sb.append(t)
                t = cpool.tile([P, m["nwin"] * GPC], bf16, tag=f"pm{b}", name=f"pm{b}sb")
                nc.sync.dma_start(out=t[:], in_=pm_d[b].ap())
                pm_sb.append(t)
                t = cpool.tile([GPC, 1], f32, tag=f"ci{b}", name=f"ci{b}sb")
                nc.sync.dma_start(out=t[:], in_=ci_d[b].ap())
                ci_sb.append(t)
                if has_bias[b]:
                    t = cpool.tile([1, D], bf16, tag=f"bias{b}", name=f"bias{b}sb")
                    nc.sync.dma_start(out=t[:], in_=bias_d[b].ap())
                    bias_sb.append(t)
                else:
                    bias_sb.append(None)
                if has_fcb[b]:
                    t = cpool.tile([1, OUT_D], f32, tag=f"fcb{b}", name=f"fcb{b}sb")
                    nc.sync.dma_start(out=t[:], in_=fcb_d[b].ap())
                    fcb_sb.append(t)
                else:
                    fcb_sb.append(None)
            for b in (0, 1):
                t = cpool.tile([P, NJ * D], bf16, tag=f"W{b}", name=f"W{b}sb")
                nc.sync.dma_start(out=t[:], in_=W_d[b].ap())
                W_sb.append(t)
                t = cpool.tile([P, NJ * OUT_D], f32, tag=f"fc{b}", name=f"fc{b}sb")
                nc.sync.dma_start(out=t[:], in_=fc_d[b].ap())
                fc_sb.append(t)
            fin_sb = cpool.tile([P, 2], f32, tag="fin", name="finsb")
            nc.sync.dma_start(out=fin_sb[:], in_=fin_d.ap())
            if has_finb:
                finb_sb = cpool.tile([1, 1], f32, tag="finb", name="finbsb")
                nc.sync.dma_start(out=finb_sb[:], in_=finb_d.ap())
            if has_bias[0] or has_bias[1]:
                ones_bf = cpool.tile([1, P], bf16, tag="ones_bf", name="ones_bf")
                nc.vector.memset(ones_bf[:], 1.0)
            if has_fcb[0] or has_fcb[1] or has_finb:
                ones8 = cpool.tile([1, GPC], f32, tag="ones8", name="ones8")
                nc.vector.memset(ones8[:], 1.0)

            repeat = int(os.environ.get("K_REPEAT", "1"))
            for _rep in range(repeat):
                wins = [(b, w) for b in (0, 1) for w in range(metas[b]["nwin"])]
                nW = len(wins)
                xg_t = [None] * nW
                xs_t = [None] * nW
                oh_t = [None] * nW
                z_t = [None] * nW
                zt_t = [None] * nW
                y_t = [None] * nW
                pool_ps = [None, None]
                y1T = [None, None]
                poolT = [None, None]

                def prefetch(i):
                    b, w = wins[i]
                    m = metas[b]
                    cw, ofs = m["cpw"][w], m["off"][w]
                    xg = xgpool.tile([P, maxcw * D], gdt, tag="xg",
                                     name=f"xg{_rep}_{b}_{w}")
                    # split across SWDGE queues: descriptor processing is
                    # per-queue rate-limited (~9 ns/desc); rotating queue
                    # pairs keep all queues busy across in-flight windows
                    ch = ((cw // 2) + 1) // 2 * 2 if cw > 2 else cw
                    splits = [(0, ch)] + ([(ch, cw)] if ch < cw else [])
                    for s, (c0, c1) in enumerate(splits):
                        nc.gpsimd.dma_gather(
                            out_ap=xg[:, c0 * D:c1 * D].rearrange(
                                "p (c f) -> p c f", f=D),
                            in_ap=xs_d[b].ap(),
                            idxs_ap=idx_sb[b][:, (ofs + c0) * 8:(ofs + c1) * 8],
                            num_idxs=(c1 - c0) * P,
                            num_idxs_reg=(c1 - c0) * P,
                            elem_size=D,
                            single_packet=False,
                            queue_num=(2 * i + s) % nq,
                        )
                    xg_t[i] = xg
                    xslf = xspool.tile([P, D], bf16, tag="xslf",
                                       name=f"xslf{_rep}_{b}_{w}")
                    nc.sync.dma_start(out=xslf[:],
                                      in_=xself_d[b].ap()[w * P:(w + 1) * P, :])
                    xs_t[i] = xslf
                    oh = ohpool.tile([P, maxcw * P], sdt, tag="oh",
                                     name=f"oh{_rep}_{b}_{w}")
                    nc.vector.tensor_tensor(
                        out=oh[:, 0:cw * P].rearrange("p (c d) -> p c d", d=P),
                        in0=dl_sb[b][:, ofs:ofs + cw].to_broadcast([P, cw, P]),
                        in1=iotam[:].rearrange("p (c d) -> p c d", c=1)
                            .to_broadcast([P, cw, P]),
                        op=Alu.is_equal)
                    oh_t[i] = oh

                def agg(i):
                    b, w = wins[i]
                    m = metas[b]
                    cw = m["cpw"][w]
                    xg, oh = xg_t[i], oh_t[i]
                    xgv = xg[:, 0:cw * D].rearrange("p (c f) -> p c f", f=D)
                    ohv = oh[:, 0:cw * P].rearrange("p (c d) -> p c d", d=P)
                    z_ps = pz.tile([P, D], f32, tag="z", name=f"z_{_rep}_{i}")
                    if dr:
                        npair = cw // 2
                        for cp in range(npair):
                            nc.tensor.matmul(
                                z_ps[:, 0:HALF], ohv[:, 2 * cp:2 * cp + 2, :],
                                xgv[:, 2 * cp:2 * cp + 2, 0:HALF],
                                start=(cp == 0),
                                stop=(cp == npair - 1 and cw % 2 == 0),
                                perf_mode=DR)
                        if cw % 2:
                            nc.tensor.matmul(
                                z_ps[:, 0:HALF], oh[:, (cw - 1) * P:cw * P],
                                xg[:, (cw - 1) * D:(cw - 1) * D + HALF],
                                start=(npair == 0), stop=True)
                    else:
                        for c in range(cw):
                            nc.tensor.matmul(
                                z_ps[:, 0:HALF], oh[:, c * P:(c + 1) * P],
                                xg[:, c * D:c * D + HALF],
                                start=(c == 0), stop=(c == cw - 1))
                    for c in range(cw):
                        rhs = xg[:, c * D + HALF:(c + 1) * D]
                        if mode == "split":
                            rhs = rhs.bitcast(e3)
                        nc.tensor.matmul(
                            z_ps[:, HALF:D], oh[:, c * P:(c + 1) * P], rhs,
                            start=(c == 0), stop=(c == cw - 1))
                    # evict: z = z_ps * dinv[dst] + xself (self-loop term), bf16
                    z = zpool.tile([P, D], bf16, tag="z", name=f"z{_rep}_{i}")
                    nc.vector.scalar_tensor_tensor(
                        out=z[:], in0=z_ps[:], scalar=dinv_sb[b][:, w:w + 1],
                        in1=xs_t[i][:], op0=Alu.mult, op1=Alu.add)
                    xs_t[i] = None
                    z_t[i] = z

                def transp(i):
                    z = z_t[i]
                    zt_ps = pzt.tile([P, D], bf16, tag="zt", name=f"zt_{_rep}_{i}")
                    for j in range(NJ):
                        nc.tensor.transpose(
                            zt_ps[:, j * P:(j + 1) * P],
                            z[:, j * P:(j + 1) * P],
                            identity_bf[:])
                    zt = ztpool.tile([P, D], bf16, tag="ztsb", name=f"ztsb{_rep}_{i}")
                    nc.scalar.copy(out=zt[:, 0:HALF], in_=zt_ps[:, 0:HALF])
                    nc.vector.tensor_copy(out=zt[:, HALF:D], in_=zt_ps[:, HALF:D])
                    zt_t[i] = zt

                def wmat(i):
                    b, w = wins[i]
                    zt = zt_t[i]
                    h_ps = ph.tile([P, D], f32, tag="h", name=f"h{_rep}_{i}")
                    for j in range(NJ):
                        lhsT = zt[:, j * P:(j + 1) * P]
                        st = j == 0
                        sp = (j == NJ - 1) and not has_bias[b]
                        nc.tensor.matmul(h_ps[:, 0:HALF], lhsT,
                                         W_sb[b][:, j * D:j * D + HALF],
                                         start=st, stop=sp)
                        nc.tensor.matmul(h_ps[:, HALF:D], lhsT,
                                         W_sb[b][:, j * D + HALF:(j + 1) * D],
                                         start=st, stop=sp)
                    if has_bias[b]:
                        nc.tensor.matmul(h_ps[:, 0:HALF], ones_bf[:],
                                         bias_sb[b][:, 0:HALF], start=False, stop=True)
                        nc.tensor.matmul(h_ps[:, HALF:D], ones_bf[:],
                                         bias_sb[b][:, HALF:D], start=False, stop=True)
                    y = ypool.tile([P, D], bf16, tag="y", name=f"y{_rep}_{i}")
                    nc.scalar.activation(out=y[:], in_=h_ps[:], func=Act.Lrelu,
                                         alpha=0.01)
                    y_t[i] = y

                def poolmat(i):
                    b, w = wins[i]
                    m = metas[b]
                    if w == 0:
                        pool_ps[b] = pp.tile([GPC, D], f32, tag="pp",
                                             name=f"pool{_rep}_{b}")
                    y = y_t[i]
                    plhsT = pm_sb[b][:, w * GPC:(w + 1) * GPC]
                    st = w == 0
                    sp = w == m["nwin"] - 1
                    nc.tensor.matmul(pool_ps[b][:, 0:HALF], plhsT, y[:, 0:HALF],
                                     start=st, stop=sp)
                    nc.tensor.matmul(pool_ps[b][:, HALF:D], plhsT, y[:, HALF:D],
                                     start=st, stop=sp)
                    y_t[i] = None
                    if sp and os.environ.get("K_ABLATE", "") != "noepi":
                        epilogue(b)

                def epilogue(b):
                    # mean-pool scale, transpose to [feat, graph], fc head
                    pacc = cpool.tile([GPC, D], f32, tag=f"pa{b}", name=f"pa{b}sb")
                    nc.vector.tensor_scalar(
                        out=pacc[:], in0=pool_ps[b][:], scalar1=ci_sb[b][:],
                        scalar2=None, op0=Alu.mult)
                    pt_ps = pp.tile([P, NJ * GPC], f32, tag="pp",
                                    name=f"pt{_rep}_{b}ps")
                    for j in range(NJ):
                        nc.tensor.transpose(
                            pt_ps[:, j * GPC:(j + 1) * GPC],
                            pacc[0:GPC, j * P:(j + 1) * P],
                            identity[0:GPC, 0:GPC])
                    t = cpool.tile([P, NJ * GPC], f32, tag=f"pT{b}", name=f"pT{b}sb")
                    nc.vector.tensor_copy(out=t[:], in_=pt_ps[:])
                    poolT[b] = t

                    h1_ps = pp.tile([P, GPC], f32, tag="pp", name=f"h1_{_rep}_{b}ps")
                    for j in range(NJ):
                        nc.tensor.matmul(
                            h1_ps[:],
                            fc_sb[b][:, j * OUT_D:(j + 1) * OUT_D],
                            poolT[b][:, j * GPC:(j + 1) * GPC],
                            start=(j == 0), stop=(j == NJ - 1) and not has_fcb[b])
                    if has_fcb[b]:
                        nc.tensor.matmul(h1_ps[:], fcb_sb[b][:],
                                         ones8[:], start=False, stop=True)
                    t = cpool.tile([P, GPC], f32, tag=f"y1T{b}", name=f"y1T{b}sb")
                    nc.scalar.activation(out=t[:], in_=h1_ps[:], func=Act.Lrelu,
                                         alpha=0.01)
                    y1T[b] = t

                    if b == 1:
                        out_ps = pp.tile([GPC, 1], f32, tag="pp",
                                         name=f"outps{_rep}")
                        nc.tensor.matmul(out_ps[:], y1T[0][:],
                                         fin_sb[:, 0:1], start=True, stop=False)
                        nc.tensor.matmul(out_ps[:], y1T[1][:],
                                         fin_sb[:, 1:2],
                                         start=False, stop=not has_finb)
                        if has_finb:
                            nc.tensor.matmul(out_ps[:], ones8[:],
                                             finb_sb[:], start=False, stop=True)
                        out_sb = cpool.tile([GPC, 1], f32, tag="out_sb",
                                            name="out_sb")
                        nc.vector.tensor_copy(out=out_sb[:], in_=out_ps[:])
                        nc.sync.dma_start(out=out_d.ap(), in_=out_sb[:])

                abl = os.environ.get("K_ABLATE", "")
                do_gather = abl != "nogather"
                do_agg = abl in ("", "agg", "now", "nogather", "nopool", "noepi")
                do_trans = abl in ("", "now", "nogather", "nopool", "noepi")
                do_w = abl in ("", "nogather", "nopool", "noepi")
                do_pool = abl in ("", "nogather", "noepi")
                do_epi = abl in ("", "nogather")
                for t in range(nW + 4):
                    if t < nW and do_gather:
                        prefetch(t)
                    elif t < nW:
                        b, w = wins[t]
                        m = metas[b]
                        cw = m["cpw"][w]
                        xg = xgpool.tile([P, maxcw * D], gdt, tag="xg",
                                         name=f"xg{_rep}_{b}_{w}")
                        xg_t[t] = xg
                        oh = ohpool.tile([P, maxcw * P], sdt, tag="oh",
                                         name=f"oh{_rep}_{b}_{w}")
                        nc.vector.tensor_tensor(
                            out=oh[:, 0:cw * P].rearrange("p (c d) -> p c d", d=P),
                            in0=dl_sb[b][:, m["off"][w]:m["off"][w] + cw]
                                .to_broadcast([P, cw, P]),
                            in1=iotam[:].rearrange("p (c d) -> p c d", c=1)
                                .to_broadcast([P, cw, P]),
                            op=Alu.is_equal)
                        oh_t[t] = oh
                    if 0 <= t - 1 < nW and do_agg:
                        agg(t - 1)
                    if 0 <= t - 2 < nW and do_trans:
                        transp(t - 2)
                    if 0 <= t - 3 < nW and do_w:
                        wmat(t - 3)
                    if 0 <= t - 4 < nW and do_pool:
                        poolmat(t - 4)
                if abl:
                    out_sb = cpool.tile([GPC, 1], f32, tag="out_sb", name="out_sb")
                    nc.vector.memset(out_sb[:], 0.0)
                    nc.sync.dma_start(out=out_d.ap(), in_=out_sb[:])

    nc.compile()
    return nc


def build_in_maps(pro1_x, pro1_edge_index, pro1_batch, pro2_x, pro2_edge_index,
                  pro2_batch, W1, b1, fc1_W, fc1_b, W2, b2, fc2_W, fc2_b,
                  final_W, final_b):
    meta0, pc0 = _prep_branch(pro1_x, pro1_edge_index, pro1_batch)
    meta1, pc1 = _prep_branch(pro2_x, pro2_edge_index, pro2_batch)

    b1 = np.asarray(b1, np.float32)
    b2 = np.asarray(b2, np.float32)
    fc1_b = np.asarray(fc1_b, np.float32)
    fc2_b = np.asarray(fc2_b, np.float32)
    final_b = np.asarray(final_b, np.float32)
    has_bias = (bool(np.any(b1)), bool(np.any(b2)))
    has_fcb = (bool(np.any(fc1_b)), bool(np.any(fc2_b)))
    has_finb = bool(np.any(final_b))

    Wr = (_reshape_w(W1), _reshape_w(W2))
    fcr = (_reshape_fc(fc1_W), _reshape_fc(fc2_W))
    fin = np.ascontiguousarray(
        np.asarray(final_W, np.float32).reshape(2, P).T)

    in_maps = []
    for c in range(NCORES):
        m = {}
        for b, pc in ((0, pc0), (1, pc1)):
            d = pc[c]
            m[f"xs{b}"] = d["xs"]
            m[f"xself{b}"] = d["xself"]
            m[f"idx{b}"] = d["idx"]
            m[f"dl{b}"] = d["dl"]
            m[f"dinv{b}"] = d["dinv"]
            m[f"pm{b}"] = d["pm"]
            m[f"ci{b}"] = d["ci"]
            m[f"W{b}"] = Wr[b]
            m[f"fc{b}"] = fcr[b]
            if has_bias[b]:
                m[f"bias{b}"] = (b1 if b == 0 else b2).reshape(1, D).astype(BF16)
            if has_fcb[b]:
                m[f"fcb{b}"] = (fc1_b if b == 0 else fc2_b).reshape(1, OUT_D)
        m["fin"] = fin
        m["iotam"] = _IOTAM
        if has_finb:
            m["finb"] = final_b.reshape(1, 1)
        in_maps.append(m)
    return meta0, meta1, (has_bias, has_fcb, has_finb), in_maps


def kernel(pro1_x, pro1_edge_index, pro1_batch, pro2_x, pro2_edge_index, pro2_batch,
           W1, b1, fc1_W, fc1_b, W2, b2, fc2_W, fc2_b, final_W, final_b):
    meta0, meta1, (has_bias, has_fcb, has_finb), in_maps = build_in_maps(
        pro1_x, pro1_edge_index, pro1_batch, pro2_x, pro2_edge_index, pro2_batch,
        W1, b1, fc1_W, fc1_b, W2, b2, fc2_W, fc2_b, final_W, final_b)

    key = (meta0["nwin"], meta0["cpw"], meta1["nwin"], meta1["cpw"],
           has_bias, has_fcb, has_finb, _mode())
    nc = _PROGRAM_CACHE.get(key)
    if nc is None:
        nc = _build_program(meta0, meta1, has_bias, has_fcb, has_finb)
        _PROGRAM_CACHE[key] = nc

    res = bass_utils.run_bass_kernel_spmd(
        nc, in_maps, core_ids=list(range(NCORES)), trace=TRACE[0])
    LAST_RESULTS[0] = res
    out = np.concatenate([res.results[c]["out"] for c in range(NCORES)], axis=0)
    return out.astype(np.float32)


# revision 24
# speedup vs baseline: 1.1250x; 1.1250x over previous
"""Trainium2 Bass kernel for a 2-branch GCN (gcn_conv -> leaky_relu -> mean-pool -> fc
head), data-parallel over graphs across 8 NeuronCores.

Math: gcn_conv(x, A, W) = (A_norm @ x) @ W + b. We aggregate raw node features first
(gather rows straight from DRAM), then do the dense 1024x1024 matmul on each core's
node shard only. dinv[src]*dinv[dst] factorizes: x rows are pre-scaled by dinv on the
host, aggregated rows are scaled by dinv[dst] on the device; the self-loop (1/deg)
folds in as a plain self-edge.

Quantization: the gathered features are fp8 to halve the dominant HBM gather
traffic. Features 0-511 are stored as fp8 e4m3 and aggregated with DoubleRow
matmuls (2 k-subtiles per instruction); features 512-1023 are fp8 e3m4 (one more
mantissa bit) aggregated at the normal rate. The one-hot lhsT is built once in
e4m3 and shared (mixed e4 lhsT x e3 rhs is supported). Everything downstream of
the aggregation (W matmul, pooling, fc heads) stays bf16/f32.

Sharding: core c owns graphs [8c, 8c+8) (batch is sorted, so its nodes are one
contiguous range). Edges are routed by dst. The per-window flow is software-
pipelined four stages deep (gather -> aggregate -> transpose -> W/pool) so the
TensorEngine never waits on DMA or PSUM evictions.
"""

import os
import sys

for _p in ("/opt/trn_rl_repo",):
    if _p not in sys.path:
        sys.path.insert(0, _p)

import numpy as np
import ml_dtypes

import concourse.bacc as bacc
import concourse.mybir as mybir
import concourse.tile as tile
from concourse import bass_utils
from concourse.masks import make_identity

N, E, B, D, OUT_D = 10000, 160000, 64, 1024, 128
NCORES = 8
GPC = B // NCORES  # graphs per core
P = 128
NJ = D // P  # feature chunks (8)
HALF = D // 2

BF16 = ml_dtypes.bfloat16
E4M3 = ml_dtypes.float8_e4m3
E3M4 = ml_dtypes.float8_e3m4

TRACE = [False]  # test.py can flip this for profiled runs
LAST_RESULTS = [None]

_IOTAM = np.ascontiguousarray(
    np.tile(np.arange(128, dtype=np.float32)[None, :], (128, 1)))


def _mode():
    return os.environ.get("K_GDT", "split")  # split | e3 | bf16


def _pack_xs(xsf):
    """[N, D] f32 (dinv-scaled) -> packed DRAM payload per K_GDT."""
    mode = _mode()
    if mode == "bf16":
        return np.ascontiguousarray(xsf.astype(BF16))
    if mode == "e3":
        return np.ascontiguousarray(xsf.astype(E3M4))
    a = xsf[:, :HALF].astype(E4M3)
    b = xsf[:, HALF:].astype(E3M4)
    packed = np.empty((N, D), np.uint8)
    packed[:, :HALF] = a.view(np.uint8)
    packed[:, HALF:] = b.view(np.uint8)
    return packed.view(E4M3)


def _prep_branch(x, edge_index, batch):
    """Host-side index preprocessing + array layout for one GCN branch.

    Returns (meta, percore) where meta = dict(nwin, cpw, off, totch) and
    percore[c] = dict of arrays named without branch suffix.
    """
    x = np.asarray(x, np.float32)
    src = np.asarray(edge_index[0], np.int64)
    dst = np.asarray(edge_index[1], np.int64)
    batch = np.asarray(batch, np.int64)

    cnt = np.bincount(batch, minlength=B)
    gstart = np.zeros(B + 1, np.int64)
    gstart[1:] = np.cumsum(cnt)

    deg = np.bincount(dst, minlength=N).astype(np.float32) + 1.0
    dinv = (1.0 / np.sqrt(deg)).astype(np.float32)
    xs = _pack_xs(x * dinv[:, None])
    # self-loop term (x[i]/deg[i]) is NOT gathered: it's contiguous rows,
    # streamed separately in bf16 and added during the z eviction
    xself_full = (x * (dinv * dinv)[:, None]).astype(BF16)

    order = np.argsort(dst, kind="stable")
    src_all = src[order]
    dst_all = dst[order]

    cores = np.arange(NCORES)
    n0 = gstart[cores * GPC]
    n1 = gstart[cores * GPC + GPC]
    e0 = np.searchsorted(dst_all, n0, side="left")
    e1 = np.searchsorted(dst_all, n1, side="left")
    nodes_c = n1 - n0
    nwin = int(np.max((nodes_c + P - 1) // P))

    wcounts = np.zeros((NCORES, nwin), np.int64)
    for c in range(NCORES):
        w = (dst_all[e0[c]:e1[c]] - n0[c]) // P
        wcounts[c] = np.bincount(w, minlength=nwin)
    cpw = np.maximum(1, (wcounts.max(axis=0) + P - 1) // P).astype(np.int64)
    off = np.zeros(nwin + 1, np.int64)
    off[1:] = np.cumsum(cpw)
    totch = int(off[-1])

    percore = []
    for c in range(NCORES):
        s_loc = src_all[e0[c]:e1[c]]
        d_loc = dst_all[e0[c]:e1[c]] - n0[c]
        w = d_loc // P
        wstart = np.zeros(nwin + 1, np.int64)
        wstart[1:] = np.cumsum(wcounts[c])
        pos_in_win = np.arange(len(d_loc)) - wstart[w]
        slot = off[w] * P + pos_in_win  # unique edge slot in [0, totch*P)

        src_pad = np.zeros(totch * P, np.int32)
        src_pad[slot] = s_loc

        dl = np.full(totch * P, -1.0, np.float32)
        dl[slot] = (d_loc % P).astype(np.float32)
        dstloc = np.ascontiguousarray(dl.reshape(totch, P).T)  # [128, totch]

        # int16 gather indices, wrapped per *window* (one dma_gather per window):
        # within a call, edge e -> [e % 16, e // 16], replicated to 128 partitions
        idx = np.zeros((16, totch * 8), np.int16)
        for wi in range(nwin):
            sw = src_pad[off[wi] * P: off[wi + 1] * P]
            idx[:, off[wi] * 8: off[wi + 1] * 8] = (
                sw.astype(np.int16).reshape(-1, 16).T
            )
        idx = np.ascontiguousarray(np.tile(idx, (8, 1)))

        ids = (n0[c] + np.arange(nwin * P)).reshape(nwin, P)
        valid = ids < n1[c]
        idc = np.minimum(ids, N - 1)
        dv = np.where(valid, dinv[idc], 0.0).astype(np.float32)
        dinv_col = np.ascontiguousarray(dv.T)  # [128, nwin]

        xself = np.where(valid.reshape(-1)[:, None], xself_full[idc.reshape(-1)],
                         np.zeros((1, D), BF16))
        xself = np.ascontiguousarray(xself.astype(BF16))  # [nwin*P, D]

        bloc = batch[idc] - c * GPC
        pm4 = np.zeros((nwin, P, GPC), np.float32)
        wi_idx, p_idx = np.nonzero(valid)
        pm4[wi_idx, p_idx, bloc[valid]] = 1.0
        pm = np.ascontiguousarray(pm4.transpose(1, 0, 2).reshape(P, nwin * GPC).astype(BF16))

        civ = (1.0 / np.maximum(cnt[c * GPC:(c + 1) * GPC], 1)).astype(np.float32)
        ci = np.ascontiguousarray(civ.reshape(GPC, 1))

        percore.append(
            dict(xs=xs, dl=dstloc, idx=idx, dinv=dinv_col, pm=pm, ci=ci,
                 xself=xself)
        )

    meta = dict(nwin=nwin, cpw=tuple(int(v) for v in cpw),
                off=tuple(int(v) for v in off), totch=totch)
    return meta, percore


def _reshape_w(W):
    # [D, D] -> [128, NJ*D]  with [p, j*D + o] = W[j*128 + p, o]
    W = np.asarray(W, np.float32)
    return np.ascontiguousarray(
        W.reshape(NJ, P, D).transpose(1, 0, 2).reshape(P, NJ * D).astype(BF16)
    )


def _reshape_fc(Wfc):
    # [D, OUT_D] -> [128, NJ*OUT_D] fp32
    Wfc = np.asarray(Wfc, np.float32)
    return np.ascontiguousarray(
        Wfc.reshape(NJ, P, OUT_D).transpose(1, 0, 2).reshape(P, NJ * OUT_D)
    )


_PROGRAM_CACHE = {}


def _build_program(meta0, meta1, has_bias, has_fcb, has_finb):
    mode = _mode()
    f32 = mybir.dt.float32
    bf16 = mybir.dt.bfloat16
    if mode == "bf16":
        gdt = sdt = bf16
        dr = False
    elif mode == "e3":
        gdt = sdt = mybir.dt.float8e3
        dr = False
    else:
        gdt = sdt = mybir.dt.float8e4
        dr = os.environ.get("K_NODR", "") != "1"
    e3 = mybir.dt.float8e3 if mode == "split" else sdt
    esz = 2 if mode == "bf16" else 1  # bytes per gathered element
    Alu = mybir.AluOpType
    Act = mybir.ActivationFunctionType
    DR = mybir.MatmulPerfMode.DoubleRow

    metas = (meta0, meta1)
    maxcw = max(max(meta0["cpw"]), max(meta1["cpw"]))
    nq = int(os.environ.get("K_NQ", "4"))
    nc = bacc.Bacc("TRN2", num_devices=NCORES, debug=False, num_swdge_queues=nq)

    # DRAM tensors
    xs_d, idx_d, dl_d, dinv_d, pm_d, ci_d, W_d, bias_d, fc_d, fcb_d = (
        [], [], [], [], [], [], [], [], [], [])
    xself_d = []
    for b in (0, 1):
        m = metas[b]
        xs_d.append(nc.dram_tensor(f"xs{b}", [N, D], gdt, kind="ExternalInput"))
        idx_d.append(nc.dram_tensor(f"idx{b}", [P, m["totch"] * 8], mybir.dt.int16,
                                    kind="ExternalInput"))
        dl_d.append(nc.dram_tensor(f"dl{b}", [P, m["totch"]], f32,
                                   kind="ExternalInput"))
        dinv_d.append(nc.dram_tensor(f"dinv{b}", [P, m["nwin"]], f32,
                                     kind="ExternalInput"))
        pm_d.append(nc.dram_tensor(f"pm{b}", [P, m["nwin"] * GPC], bf16,
                                   kind="ExternalInput"))
        xself_d.append(nc.dram_tensor(f"xself{b}", [m["nwin"] * P, D], bf16,
                                      kind="ExternalInput"))
        ci_d.append(nc.dram_tensor(f"ci{b}", [GPC, 1], f32, kind="ExternalInput"))
        W_d.append(nc.dram_tensor(f"W{b}", [P, NJ * D], bf16, kind="ExternalInput"))
        fc_d.append(nc.dram_tensor(f"fc{b}", [P, NJ * OUT_D], f32,
                                   kind="ExternalInput"))
        bias_d.append(nc.dram_tensor(f"bias{b}", [1, D], bf16, kind="ExternalInput")
                      if has_bias[b] else None)
        fcb_d.append(nc.dram_tensor(f"fcb{b}", [1, OUT_D], f32, kind="ExternalInput")
                     if has_fcb[b] else None)
    fin_d = nc.dram_tensor("fin", [P, 2], f32, kind="ExternalInput")
    iota_d = nc.dram_tensor("iotam", [P, P], f32, kind="ExternalInput")
    finb_d = nc.dram_tensor("finb", [1, 1], f32, kind="ExternalInput") if has_finb else None
    out_d = nc.dram_tensor("out", [GPC, 1], f32, kind="ExternalOutput")

    with tile.TileContext(nc) as tc:
        with (
            tc.tile_pool(name="const", bufs=1) as cpool,
            tc.tile_pool(name="xg", bufs=4) as xgpool,
            tc.tile_pool(name="ohp", bufs=4) as ohpool,
            tc.tile_pool(name="zsb", bufs=2) as zpool,
            tc.tile_pool(name="ztsb", bufs=2) as ztpool,
            tc.tile_pool(name="ysb", bufs=3) as ypool,
            tc.tile_pool(name="xslf", bufs=4) as xspool,
            tc.tile_pool(name="pz", bufs=1, space="PSUM") as pz,
            tc.tile_pool(name="pzt", bufs=2, space="PSUM") as pzt,
            tc.tile_pool(name="ph", bufs=1, space="PSUM") as ph,
            tc.tile_pool(name="pp", bufs=1, space="PSUM") as pp,
        ):
            identity = cpool.tile([P, P], f32, tag="ident", name="ident")
            make_identity(nc, identity[:])
            identity_bf = cpool.tile([P, P], bf16, tag="identbf", name="identbf")
            nc.vector.tensor_copy(out=identity_bf[:], in_=identity[:])
            iotam = cpool.tile([P, P], f32, tag="iotam", name="iotam")
            nc.sync.dma_start(out=iotam[:], in_=iota_d.ap())

            # resident small/medium tensors
            idx_sb, dl_sb, dinv_sb, pm_sb, ci_sb, bias_sb, fcb_sb = (
                [], [], [], [], [], [], [])
            W_sb, fc_sb = [], []
            for b in (0, 1):
                m = metas[b]
                t = cpool.tile([P, m["totch"] * 8], mybir.dt.int16, tag=f"idx{b}",
                               name=f"idx{b}sb")
                nc.sync.dma_start(out=t[:], in_=idx_d[b].ap())
                idx_sb.append(t)
                t = cpool.tile([P, m["totch"]], f32, tag=f"dl{b}", name=f"dl{b}sb")
                nc.sync.dma_start(out=t[:], in_=dl_d[b].ap())
                dl_sb.append(t)
                t = cpool.tile([P, m["nwin"]], f32, tag=f"dinv{b}", name=f"dinv{b}sb")
                nc.sync.dma_start(out=t[:], in_=dinv_d[b].ap())
                dinv_sb.append(t)
                t = cpool.tile([P, m["nwin"] * GPC], bf16, tag=f"pm{b}", name=f"pm{b}sb")
                nc.sync.dma_start(out=t[:], in_=pm_d[b].ap())
                pm_sb.append(t)
                t = cpool.tile([GPC, 1], f32, tag=f"ci{b}", name=f"ci{b}sb")
                nc.sync.dma_start(out=t[:], in_=ci_d[b].ap())
                ci_sb.append(t)
                if has_bias[b]:
                    t = cpool.tile([1, D], bf16, tag=f"bias{b}", name=f"bias{b}sb")
                    nc.sync.dma_start(out=t[:], in_=bias_d[b].ap())
                    bias_sb.append(t)
                else:
                    bias_sb.append(None)
                if has_fcb[b]:
                    t = cpool.tile([1, OUT_D], f32, tag=f"fcb{b}", name=f"fcb{b}sb")
                    nc.sync.dma_start(out=t[:], in_=fcb_d[b].ap())
                    fcb_sb.append(t)
                else:
                    fcb_sb.append(None)
            for b in (0, 1):
                t = cpool.tile([P, NJ * D], bf16, tag=f"W{b}", name=f"W{b}sb")
                nc.sync.dma_start(out=t[:], in_=W_d[b].ap())
                W_sb.append(t)
                t = cpool.tile([P, NJ * OUT_D], f32, tag=f"fc{b}", name=f"fc{b}sb")
                nc.sync.dma_start(out=t[:], in_=fc_d[b].ap())
                fc_sb.append(t)
            fin_sb = cpool.tile([P, 2], f32, tag="fin", name="finsb")
            nc.sync.dma_start(out=fin_sb[:], in_=fin_d.ap())
            if has_finb:
                finb_sb = cpool.tile([1, 1], f32, tag="finb", name="finbsb")
                nc.sync.dma_start(out=finb_sb[:], in_=finb_d.ap())
            if has_bias[0] or has_bias[1]:
                ones_bf = cpool.tile([1, P], bf16, tag="ones_bf", name="ones_bf")
                nc.vector.memset(ones_bf[:], 1.0)
            if has_fcb[0] or has_fcb[1] or has_finb:
                ones8 = cpool.tile([1, GPC], f32, tag="ones8", name="ones8")
                nc.vector.memset(ones8[:], 1.0)

            repeat = int(os.environ.get("K_REPEAT", "1"))
            for _rep in range(repeat):
                wins = [(b, w) for b in (0, 1) for w in range(metas[b]["nwin"])]
                nW = len(wins)
                xg_t = [None] * nW
                xs_t = [None] * nW
                oh_t = [None] * nW
                z_t = [None] * nW
                zt_t = [None] * nW
                y_t = [None] * nW
                pool_ps = [None, None]
                y1T = [None, None]
                poolT = [None, None]

                def prefetch(i):
                    b, w = wins[i]
                    m = metas[b]
                    cw, ofs = m["cpw"][w], m["off"][w]
                    xg = xgpool.tile([P, maxcw * D], gdt, tag="xg",
                                     name=f"xg{_rep}_{b}_{w}")
                    # split across SWDGE queues: descriptor processing is
                    # per-queue rate-limited (~9 ns/desc); rotating queue
                    # pairs keep all queues busy across in-flight windows
                    ch = ((cw // 2) + 1) // 2 * 2 if cw > 2 else cw
                    splits = [(0, ch)] + ([(ch, cw)] if ch < cw else [])
                    for s, (c0, c1) in enumerate(splits):
                        nc.gpsimd.dma_gather(
                            out_ap=xg[:, c0 * D:c1 * D].rearrange(
                                "p (c f) -> p c f", f=D),
                            in_ap=xs_d[b].ap(),
                            idxs_ap=idx_sb[b][:, (ofs + c0) * 8:(ofs + c1) * 8],
                            num_idxs=(c1 - c0) * P,
                            num_idxs_reg=(c1 - c0) * P,
                            elem_size=D,
                            single_packet=False,
                            queue_num=(2 * i + s) % nq,
                        )
                    xg_t[i] = xg
                    xslf = xspool.tile([P, D], bf16, tag="xslf",
                                       name=f"xslf{_rep}_{b}_{w}")
                    nc.sync.dma_start(out=xslf[:],
                                      in_=xself_d[b].ap()[w * P:(w + 1) * P, :])
                    xs_t[i] = xslf
                    oh = ohpool.tile([P, maxcw * P], sdt, tag="oh",
                                     name=f"oh{_rep}_{b}_{w}")
                    nc.vector.tensor_tensor(
                        out=oh[:, 0:cw * P].rearrange("p (c d) -> p c d", d=P),
                        in0=dl_sb[b][:, ofs:ofs + cw].to_broadcast([P, cw, P]),
                        in1=iotam[:].rearrange("p (c d) -> p c d", c=1)
                            .to_broadcast([P, cw, P]),
                        op=Alu.is_equal)
                    oh_t[i] = oh

                def agg(i):
                    b, w = wins[i]
                    m = metas[b]
                    cw = m["cpw"][w]
                    xg, oh = xg_t[i], oh_t[i]
                    xgv = xg[:, 0:cw * D].rearrange("p (c f) -> p c f", f=D)
                    ohv = oh[:, 0:cw * P].rearrange("p (c d) -> p c d", d=P)
                    z_ps = pz.tile([P, D], f32, tag="z", name=f"z_{_rep}_{i}")
                    if dr:
                        npair = cw // 2
                        for cp in range(npair):
                            nc.tensor.matmul(
                                z_ps[:, 0:HALF], ohv[:, 2 * cp:2 * cp + 2, :],
                                xgv[:, 2 * cp:2 * cp + 2, 0:HALF],
                                start=(cp == 0),
                                stop=(cp == npair - 1 and cw % 2 == 0),
                                perf_mode=DR)
                        if cw % 2:
                            nc.tensor.matmul(
                                z_ps[:, 0:HALF], oh[:, (cw - 1) * P:cw * P],
                                xg[:, (cw - 1) * D:(cw - 1) * D + HALF],
                                start=(npair == 0), stop=True)
                    else:
                        for c in range(cw):
                            nc.tensor.matmul(
                                z_ps[:, 0:HALF], oh[:, c * P:(c + 1) * P],
                                xg[:, c * D:c * D + HALF],
                                start=(c == 0), stop=(c == cw - 1))
                    for c in range(cw):
                        rhs = xg[:, c * D + HALF:(c + 1) * D]
                        if mode == "split":
                            rhs = rhs.bitcast(e3)
                        nc.tensor.matmul(
                            z_ps[:, HALF:D], oh[:, c * P:(c + 1) * P], rhs,
                            start=(c == 0), stop=(c == cw - 1))
                    # evict: z = z_ps * dinv[dst] + xself (self-loop term), bf16.
                    # Two half-ops so the e4 half's eviction overlaps the e3
                    # aggregation matmuls (subtile deps track column ranges).
                    z = zpool.tile([P, D], bf16, tag="z", name=f"z{_rep}_{i}")
                    nc.vector.scalar_tensor_tensor(
                        out=z[:, 0:HALF], in0=z_ps[:, 0:HALF],
                        scalar=dinv_sb[b][:, w:w + 1],
                        in1=xs_t[i][:, 0:HALF], op0=Alu.mult, op1=Alu.add)
                    nc.vector.scalar_tensor_tensor(
                        out=z[:, HALF:D], in0=z_ps[:, HALF:D],
                        scalar=dinv_sb[b][:, w:w + 1],
                        in1=xs_t[i][:, HALF:D], op0=Alu.mult, op1=Alu.add)
                    xs_t[i] = None
                    z_t[i] = z

                def transp(i):
                    z = z_t[i]
                    zt_ps = pzt.tile([P, D], bf16, tag="zt", name=f"zt_{_rep}_{i}")
                    for j in range(NJ):
                        nc.tensor.transpose(
                            zt_ps[:, j * P:(j + 1) * P],
                            z[:, j * P:(j + 1) * P],
                            identity_bf[:])
                    zt = ztpool.tile([P, D], bf16, tag="ztsb", name=f"ztsb{_rep}_{i}")
                    nc.scalar.copy(out=zt[:, 0:HALF], in_=zt_ps[:, 0:HALF])
                    nc.vector.tensor_copy(out=zt[:, HALF:D], in_=zt_ps[:, HALF:D])
                    zt_t[i] = zt

                def wmat(i):
                    b, w = wins[i]
                    zt = zt_t[i]
                    h_ps = ph.tile([P, D], f32, tag="h", name=f"h{_rep}_{i}")
                    for j in range(NJ):
                        lhsT = zt[:, j * P:(j + 1) * P]
                        st = j == 0
                        sp = (j == NJ - 1) and not has_bias[b]
                        nc.tensor.matmul(h_ps[:, 0:HALF], lhsT,
                                         W_sb[b][:, j * D:j * D + HALF],
                                         start=st, stop=sp)
                        nc.tensor.matmul(h_ps[:, HALF:D], lhsT,
                                         W_sb[b][:, j * D + HALF:(j + 1) * D],
                                         start=st, stop=sp)
                    if has_bias[b]:
                        nc.tensor.matmul(h_ps[:, 0:HALF], ones_bf[:],
                                         bias_sb[b][:, 0:HALF], start=False, stop=True)
                        nc.tensor.matmul(h_ps[:, HALF:D], ones_bf[:],
                                         bias_sb[b][:, HALF:D], start=False, stop=True)
                    y = ypool.tile([P, D], bf16, tag="y", name=f"y{_rep}_{i}")
                    nc.scalar.activation(out=y[:], in_=h_ps[:], func=Act.Lrelu,
                                         alpha=0.01)
                    y_t[i] = y

                def poolmat(i):
                    b, w = wins[i]
                    m = metas[b]
                    if w == 0:
                        pool_ps[b] = pp.tile([GPC, D], f32, tag="pp",
                                             name=f"pool{_rep}_{b}")
                    y = y_t[i]
                    plhsT = pm_sb[b][:, w * GPC:(w + 1) * GPC]
                    st = w == 0
                    sp = w == m["nwin"] - 1
                    nc.tensor.matmul(pool_ps[b][:, 0:HALF], plhsT, y[:, 0:HALF],
                                     start=st, stop=sp)
                    nc.tensor.matmul(pool_ps[b][:, HALF:D], plhsT, y[:, HALF:D],
                                     start=st, stop=sp)
                    y_t[i] = None
                    if sp and os.environ.get("K_ABLATE", "") != "noepi":
                        epilogue(b)

                def epilogue(b):
                    # mean-pool scale, transpose to [feat, graph], fc head
                    pacc = cpool.tile([GPC, D], f32, tag=f"pa{b}", name=f"pa{b}sb")
                    nc.vector.tensor_scalar(
                        out=pacc[:], in0=pool_ps[b][:], scalar1=ci_sb[b][:],
                        scalar2=None, op0=Alu.mult)
                    pt_ps = pp.tile([P, NJ * GPC], f32, tag="pp",
                                    name=f"pt{_rep}_{b}ps")
                    for j in range(NJ):
                        nc.tensor.transpose(
                            pt_ps[:, j * GPC:(j + 1) * GPC],
                            pacc[0:GPC, j * P:(j + 1) * P],
                            identity[0:GPC, 0:GPC])
                    t = cpool.tile([P, NJ * GPC], f32, tag=f"pT{b}", name=f"pT{b}sb")
                    nc.vector.tensor_copy(out=t[:], in_=pt_ps[:])
                    poolT[b] = t

                    h1_ps = pp.tile([P, GPC], f32, tag="pp", name=f"h1_{_rep}_{b}ps")
                    for j in range(NJ):
                        nc.tensor.matmul(
                            h1_ps[:],
                            fc_sb[b][:, j * OUT_D:(j + 1) * OUT_D],
                            poolT[b][:, j * GPC:(j + 1) * GPC],
                            start=(j == 0), stop=(j == NJ - 1) and not has_fcb[b])
                    if has_fcb[b]:
                        nc.tensor.matmul(h1_ps[:], fcb_sb[b][:],
                                         ones8[:], start=False, stop=True)
                    t = cpool.tile([P, GPC], f32, tag=f"y1T{b}", name=f"y1T{b}sb")
                    nc.scalar.activation(out=t[:], in_=h1_ps[:], func=Act.Lrelu,
                                         alpha=0.01)
                    y1T[b] = t

                    if b == 1:
                        out_ps = pp.tile([GPC, 1], f32, tag="pp",
                                         name=f"outps{_rep}")
                        nc.tensor.matmul(out_ps[:], y1T[0][:],
                                         fin_sb[:, 0:1], start=True, stop=False)
                        nc.tensor.matmul(out_ps[:], y1T[1][:],
                                         fin_sb[:, 1:2],
                                         start=False, stop=not has_finb)
                        if has_finb:
                            nc.tensor.matmul(out_ps[:], ones8[:],
                                             finb_sb[:], start=False, stop=True)
                        out_sb = cpool.tile([GPC, 1], f32, tag="out_sb",
                                            name="out_sb")
                        nc.vector.tensor_copy(out=out_sb[:], in_=out_ps[:])
                        nc.sync.dma_start(out=out_d.ap(), in_=out_sb[:])

                abl = os.environ.get("K_ABLATE", "")
                do_gather = abl != "nogather"
                do_agg = abl in ("", "agg", "now", "nogather", "nopool", "noepi")
                do_trans = abl in ("", "now", "nogather", "nopool", "noepi")
                do_w = abl in ("", "nogather", "nopool", "noepi")
                do_pool = abl in ("", "nogather", "noepi")
                do_epi = abl in ("", "nogather")
                for t in range(nW + 4):
                    if t < nW and do_gather:
                        prefetch(t)
                    elif t < nW:
                        b, w = wins[t]
                        m = metas[b]
                        cw = m["cpw"][w]
                        xg = xgpool.tile([P, maxcw * D], gdt, tag="xg",
                                         name=f"xg{_rep}_{b}_{w}")
                        xg_t[t] = xg
                        oh = ohpool.tile([P, maxcw * P], sdt, tag="oh",
                                         name=f"oh{_rep}_{b}_{w}")
                        nc.vector.tensor_tensor(
                            out=oh[:, 0:cw * P].rearrange("p (c d) -> p c d", d=P),
                            in0=dl_sb[b][:, m["off"][w]:m["off"][w] + cw]
                                .to_broadcast([P, cw, P]),
                            in1=iotam[:].rearrange("p (c d) -> p c d", c=1)
                                .to_broadcast([P, cw, P]),
                            op=Alu.is_equal)
                        oh_t[t] = oh
                    if 0 <= t - 1 < nW and do_agg:
                        agg(t - 1)
                    if 0 <= t - 2 < nW and do_trans:
                        transp(t - 2)
                    if 0 <= t - 3 < nW and do_w:
                        wmat(t - 3)
                    if 0 <= t - 4 < nW and do_pool:
                        poolmat(t - 4)
                if abl:
                    out_sb = cpool.tile([GPC, 1], f32, tag="out_sb", name="out_sb")
                    nc.vector.memset(out_sb[:], 0.0)
                    nc.sync.dma_start(out=out_d.ap(), in_=out_sb[:])

    nc.compile()
    return nc


def build_in_maps(pro1_x, pro1_edge_index, pro1_batch, pro2_x, pro2_edge_index,
                  pro2_batch, W1, b1, fc1_W, fc1_b, W2, b2, fc2_W, fc2_b,
                  final_W, final_b):
    meta0, pc0 = _prep_branch(pro1_x, pro1_edge_index, pro1_batch)
    meta1, pc1 = _prep_branch(pro2_x, pro2_edge_index, pro2_batch)

    b1 = np.asarray(b1, np.float32)
    b2 = np.asarray(b2, np.float32)
    fc1_b = np.asarray(fc1_b, np.float32)
    fc2_b = np.asarray(fc2_b, np.float32)
    final_b = np.asarray(final_b, np.float32)
    has_bias = (bool(np.any(b1)), bool(np.any(b2)))
    has_fcb = (bool(np.any(fc1_b)), bool(np.any(fc2_b)))
    has_finb = bool(np.any(final_b))

    Wr = (_reshape_w(W1), _reshape_w(W2))
    fcr = (_reshape_fc(fc1_W), _reshape_fc(fc2_W))
    fin = np.ascontiguousarray(
        np.asarray(final_W, np.float32).reshape(2, P).T)

    in_maps = []
    for c in range(NCORES):
        m = {}
        for b, pc in ((0, pc0), (1, pc1)):
            d = pc[c]
            m[f"xs{b}"] = d["xs"]
            m[f"xself{b}"] = d["xself"]
            m[f"idx{b}"] = d["idx"]
            m[f"dl{b}"] = d["dl"]
            m[f"dinv{b}"] = d["dinv"]
            m[f"pm{b}"] = d["pm"]
            m[f"ci{b}"] = d["ci"]
            m[f"W{b}"] = Wr[b]
            m[f"fc{b}"] = fcr[b]
            if has_bias[b]:
                m[f"bias{b}"] = (b1 if b == 0 else b2).reshape(1, D).astype(BF16)
            if has_fcb[b]:
                m[f"fcb{b}"] = (fc1_b if b == 0 else fc2_b).reshape(1, OUT_D)
        m["fin"] = fin
        m["iotam"] = _IOTAM
        if has_finb:
            m["finb"] = final_b.reshape(1, 1)
        in_maps.append(m)
    return meta0, meta1, (has_bias, has_fcb, has_finb), in_maps


def kernel(pro1_x, pro1_edge_index, pro1_batch, pro2_x, pro2_edge_index, pro2_batch,
           W1, b1, fc1_W, fc1_b, W2, b2, fc2_W, fc2_b, final_W, final_b):
    meta0, meta1, (has_bias, has_fcb, has_finb), in_maps = build_in_maps(
        pro1_x, pro1_edge_index, pro1_batch, pro2_x, pro2_edge_index, pro2_batch,
        W1, b1, fc1_W, fc1_b, W2, b2, fc2_W, fc2_b, final_W, final_b)

    key = (meta0["nwin"], meta0["cpw"], meta1["nwin"], meta1["cpw"],
           has_bias, has_fcb, has_finb, _mode())
    nc = _PROGRAM_CACHE.get(key)
    if nc is None:
        nc = _build_program(meta0, meta1, has_bias, has_fcb, has_finb)
        _PROGRAM_CACHE[key] = nc

    res = bass_utils.run_bass_kernel_spmd(
        nc, in_maps, core_ids=list(range(NCORES)), trace=TRACE[0])
    LAST_RESULTS[0] = res
    out = np.concatenate([res.results[c]["out"] for c in range(NCORES)], axis=0)
    return out.astype(np.float32)
